# revision 23
# baseline (speedup 1.0000x reference)
"""3-layer GAT (GATConv x3 + log_softmax) on 8 trn2 NeuronCores.

Strategy: 1-D node partition (6250 nodes/core). Edges live on the core that
owns their destination node, sorted by dst, packed into fixed windows of 128
consecutive dst nodes x K edge-tiles of 128 edges (K sized to the max window
in-degree). Fixed windows make every non-gather transfer a direct DMA; the
only indirect DMA is one batched h[src] gather per window (128*K rows).
Segment softmax + scatter-add are one-hot selection matmuls in PSUM. Layer
outputs are exchanged with shared-output AllGathers (node phase is sharded
too); al_d stays core-local (dst-partitioned).
"""
import numpy as np
import ml_dtypes

import concourse.bass as bass
import concourse.mybir as mybir
import concourse.tile as tile
from concourse.bass_utils import run_bass_kernel_spmd

BF = ml_dtypes.bfloat16
N = 50000
NC = 8
SHARD = N // NC            # 6250
H, C = 8, 64
F = H * C                  # 512
C3 = 5
F3 = H * C3                # 40
G = 4                      # tiles per inner group
W = (SHARD + 127) // 128   # 49 windows of 128 consecutive dst nodes
NPAD = W * 128             # 6272 padded shard rows
USE_DG = True              # one dma_gather per chunk instead of per-tile
DG_CH = 4                  # tiles per dma_gather chunk (512 idx)
RWG = 640                  # padded bf16 row for %256B dma_gather (512+8+pad)
RWG3 = 64                  # padded f32 row for layer-3 (256B)
NEG_SLOPE = 0.2
DT_BF = mybir.dt.bfloat16
DT_F32 = mybir.dt.float32
DT_I32 = mybir.dt.int32
AF = mybir.ActivationFunctionType
ALU = mybir.AluOpType


def _split_drain_waits(nc, max_waits=1):
    # walrus on this toolchain rejects instructions carrying more than a few
    # sync waits; keep <=max_waits per instruction, move extras onto NoOps
    # inserted just before (same engine -> executes first, semantics kept).
    ctr = 0
    for f in nc.m.functions:
        for blk in f.blocks:
            new_list = []
            for ins in blk.instructions:
                if ins.sync_info and \
                        len(ins.sync_info.on_wait) > max_waits:
                    waits = list(ins.sync_info.on_wait)
                    keep, extra = waits[:max_waits], waits[max_waits:]
                    for w in extra:
                        ctr += 1
                        new_list.append(mybir.InstNoOp(
                            name=f"drainfix-{ctr}", engine=ins.engine,
                            ins=[], outs=[],
                            sync_info=mybir.SyncInfo(on_wait=[w], on_update=[])))
                    ins.sync_info.on_wait = keep
                new_list.append(ins)
            blk.instructions[:] = new_list


def _bcast(ap, ap_list):
    """Build an AP over ap's tensor with explicit [step, count] dims."""
    return bass.AP(ap.tensor, ap.offset, ap_list)


def host_prep(edge_index):
    """Assign edges to dst-owner cores, sort by dst, pack into fixed windows
    of 128 consecutive dst nodes; K = edge tiles per window (max in-degree
    driven). Returns per-core esrc [128, T] and dstrow [128, T] (bf16)."""
    # the synthetic self-loop the reference adds per node is handled in the
    # window close from local rows; data edges (incl. any src==dst) stay here
    src = np.ascontiguousarray(edge_index[0])
    dst = np.ascontiguousarray(edge_index[1])
    order = np.argsort(dst, kind="stable")
    src, dst = src[order], dst[order]
    percore = []
    kmax = 1
    for c in range(NC):
        lo, hi = c * SHARD, (c + 1) * SHARD
        m0 = np.searchsorted(dst, lo, "left")
        m1 = np.searchsorted(dst, hi, "left")
        s_c, d_c = src[m0:m1], dst[m0:m1] - lo          # dst local [0, SHARD)
        counts = np.bincount(d_c, minlength=NPAD)
        starts = np.concatenate([[0], np.cumsum(counts)])
        wcnt = starts[128::128].copy()
        wcnt[1:] -= starts[128:-128:128]
        kmax = max(kmax, int((wcnt.max() + 127) // 128))
        percore.append((s_c, d_c, starts))
    K = int(kmax)
    T = W * K
    cap = K * 128
    esrc = np.zeros((NC, T, 128), np.int32)
    dstrow = np.full((NC, T, 128), 999.0, np.float32)
    for c, (s_c, d_c, starts) in enumerate(percore):
        for w in range(W):
            e0, e1 = starts[w * 128], starts[min((w + 1) * 128, SHARD)]
            ne = e1 - e0
            t0 = w * K
            flat_s = esrc[c, t0:t0 + K].reshape(-1)
            flat_r = dstrow[c, t0:t0 + K].reshape(-1)
            flat_s[:ne] = s_c[e0:e1]
            flat_r[:ne] = (d_c[e0:e1] - w * 128).astype(np.float32)
    return (np.ascontiguousarray(esrc.transpose(0, 2, 1)),
            np.ascontiguousarray(dstrow.transpose(0, 2, 1)),
            np.ascontiguousarray(dstrow).astype(BF), K, T)


def blockdiag(a):
    """[H, c] head vectors -> [H*c, H] block diagonal (placement only)."""
    Hh, cc = a.shape
    out = np.zeros((Hh * cc, Hh), np.float32)
    for h in range(Hh):
        out[h * cc:(h + 1) * cc, h] = a[h]
    return out


def chunk_rows(m, p=128):
    """[R, C] -> [ceil(R/p), p, C] zero-padded."""
    R, Cc = m.shape
    n = (R + p - 1) // p
    out = np.zeros((n, p, Cc), m.dtype)
    for i in range(n):
        out[i, :min(p, R - i * p)] = m[i * p:(i + 1) * p]
    return out


def build_program(K, T, sim_mode=False):
    nc = bass.Bass("TRN2")
    P = {}
    def par(name, shape, dt):
        P[name] = nc.declare_dram_parameter(name, list(shape), dt, isOutput=False)
        return P[name]

    par("xTs", [12, SHARD], DT_F32)
    par("W1", [12, F], DT_F32)
    par("W1Tc", [4, 128, 12], DT_F32)
    par("Wa1", [4, 128, 16], DT_F32)
    par("W2c", [4, 128, F], DT_F32)
    par("W2Tc", [4, 128, F], DT_F32)
    par("Wa2", [4, 128, 16], DT_F32)
    par("W3c", [4, 128, F3], DT_F32)
    par("W3T", [F3, F], DT_F32)
    par("Wa3", [F3, 16], DT_F32)
    par("b1t", [128, F], DT_F32)
    par("b2t", [128, F], DT_F32)
    par("b3t", [128, C3], DT_F32)
    par("esrc", [128, T], DT_I32)
    par("dstrow", [128, T], DT_F32)
    par("dstrowT", [T, 128], DT_BF)
    OUT = nc.declare_dram_parameter("out", [SHARD, C3], DT_F32, isOutput=True)
    par("tick", [128, 1], DT_F32)
    TOCK = nc.declare_dram_parameter("tock", [128, 1], DT_F32, isOutput=True)

    with tile.TileContext(nc) as tc:
        with (
            tc.tile_pool(name="const", bufs=1) as cp,
            tc.tile_pool(name="sbuf", bufs=3) as sb,
            tc.tile_pool(name="stage", bufs=3) as stg,
            tc.tile_pool(name="hgp", bufs=2) as hgp,
            tc.tile_pool(name="pbig", bufs=3, space="PSUM") as p_A,
            tc.tile_pool(name="psB", bufs=2, space="PSUM") as p_B,
            tc.tile_pool(name="ptr", bufs=1, space="PSUM") as p_T,
            tc.tile_pool(name="dram", bufs=1, space="DRAM") as dr,
        ):
            # ---------------- constants / weights ----------------
            ident = cp.tile([128, 128], DT_F32)
            from concourse.masks import make_identity
            make_identity(nc, ident[:])
            ident_bf = cp.tile([128, 128], DT_BF)
            nc.vector.tensor_copy(out=ident_bf[:], in_=ident[:])
            iota_i = cp.tile([128, 128], DT_I32)
            nc.gpsimd.iota(iota_i[:], pattern=[[1, 128]], base=0, channel_multiplier=0)
            iota_b = cp.tile([128, 128], DT_BF)
            nc.vector.tensor_copy(out=iota_b[:], in_=iota_i[:])
            iotaP_i = cp.tile([128, 1], DT_I32)
            nc.gpsimd.iota(iotaP_i[:], pattern=[[1, 1]], base=0, channel_multiplier=1)
            iotaP_b = cp.tile([128, 1], DT_F32)
            nc.vector.tensor_copy(out=iotaP_b[:], in_=iotaP_i[:])

            t_esrc = cp.tile([128, T], DT_I32)
            nc.sync.dma_start(out=t_esrc[:], in_=P["esrc"][:])
            t_drow = cp.tile([128, T], DT_F32)
            nc.sync.dma_start(out=t_drow[:], in_=P["dstrow"][:])
            t_b1 = cp.tile([128, F], DT_F32)
            nc.sync.dma_start(out=t_b1[:], in_=P["b1t"][:])
            t_b2 = cp.tile([128, F], DT_F32)
            nc.sync.dma_start(out=t_b2[:], in_=P["b2t"][:])
            t_b3 = cp.tile([128, C3], DT_F32)
            nc.sync.dma_start(out=t_b3[:], in_=P["b3t"][:])

            # bf16 weight copies (cast during DMA on gpsimd)
            w1 = cp.tile([12, F], DT_BF)
            nc.gpsimd.dma_start(out=w1[:], in_=P["W1"][:])
            w1T = cp.tile([128, 4, 12], DT_BF)
            w2 = cp.tile([128, 4, F], DT_BF)
            w2T = cp.tile([128, 4, F], DT_BF)
            w3 = cp.tile([128, 4, F3], DT_BF)
            wa1 = cp.tile([128, 4, 16], DT_BF)
            wa2 = cp.tile([128, 4, 16], DT_BF)
            for ch in range(4):
                nc.gpsimd.dma_start(out=w1T[:, ch, :], in_=P["W1Tc"][ch])
                nc.gpsimd.dma_start(out=w2[:, ch, :], in_=P["W2c"][ch])
                nc.gpsimd.dma_start(out=w2T[:, ch, :], in_=P["W2Tc"][ch])
                nc.gpsimd.dma_start(out=w3[:, ch, :], in_=P["W3c"][ch])
                nc.gpsimd.dma_start(out=wa1[:, ch, :], in_=P["Wa1"][ch])
                nc.gpsimd.dma_start(out=wa2[:, ch, :], in_=P["Wa2"][ch])
            w3T = cp.tile([F3, F], DT_BF)
            nc.gpsimd.dma_start(out=w3T[:], in_=P["W3T"][:])
            wa3 = cp.tile([F3, 16], DT_BF)
            nc.gpsimd.dma_start(out=wa3[:], in_=P["Wa3"][:])

            # fused attention projections  WWa_l = W_l @ Wa_l  -> [Fin_l, 16]
            wwa1 = cp.tile([12, 16], DT_BF)
            ps = p_T.tile([12, 16], DT_F32, space="PSUM", tag="pan")
            for ch in range(4):
                nc.tensor.matmul(ps[:], lhsT=w1T[:, ch, :], rhs=wa1[:, ch, :],
                                 start=(ch == 0), stop=(ch == 3))
            nc.vector.tensor_copy(out=wwa1[:], in_=ps[:])
            wwa2 = cp.tile([128, 4, 16], DT_BF)
            for fc in range(4):
                ps = p_T.tile([128, 16], DT_F32, space="PSUM", tag="pan")
                for ch in range(4):
                    nc.tensor.matmul(
                        ps[:], lhsT=w2T[:, ch, bass.ts(fc, 128)],
                        rhs=wa2[:, ch, :], start=(ch == 0), stop=(ch == 3))
                nc.vector.tensor_copy(out=wwa2[:, fc, :], in_=ps[:])
            wwa3 = cp.tile([128, 4, 16], DT_BF)
            for fc in range(4):
                ps = p_T.tile([128, 16], DT_F32, space="PSUM", tag="pan")
                nc.tensor.matmul(ps[:], lhsT=w3T[:, bass.ts(fc, 128)], rhs=wa3[:],
                                 start=True, stop=True)
                nc.vector.tensor_copy(out=wwa3[:, fc, :], in_=ps[:])

            # ---------------- DRAM internals ----------------
            OUTI = dr.tile([NPAD, C3], DT_F32)
            exch1 = dr.tile([NPAD, F + 8], DT_BF)      # h1 | al_s1 (own shard)
            ALD1 = dr.tile([NPAD, 16], DT_BF)
            Hf1 = dr.tile([N, F + 8], DT_BF, addr_space="Shared")
            exch2 = dr.tile([NPAD, F + 8], DT_BF)
            ALD2 = dr.tile([NPAD, 16], DT_BF)
            Hf2 = dr.tile([N, F + 8], DT_BF, addr_space="Shared")
            exch3 = dr.tile([NPAD, F3 + 16], DT_F32)   # h3 | al_s3 | al_d3
            H3f = dr.tile([N, F3 + 16], DT_F32, addr_space="Shared")

            # ---------------- layer-1 node phase (own shard only) -----------
            # each core receives its own pre-sliced xTs input
            xsh = cp.tile([12, NPAD], DT_BF)
            nc.vector.memset(xsh[:], 0.0)
            nc.gpsimd.dma_start(out=xsh[:, 0:SHARD], in_=P["xTs"][:])

            for t in range(W):
                lhs = xsh[:, t * 128:(t + 1) * 128]
                ph = p_A.tile([128, F], DT_F32, space="PSUM", tag="pbig")
                nc.tensor.matmul(ph[:], lhsT=lhs, rhs=w1[:], start=True, stop=True)
                pa = p_T.tile([128, 16], DT_F32, space="PSUM", tag="pan")
                nc.tensor.matmul(pa[:], lhsT=lhs, rhs=wwa1[:], start=True, stop=True)
                hstage = stg.tile([128, F + 8], DT_BF, tag="h1s")
                nc.vector.tensor_copy(out=hstage[:, :F], in_=ph[:])
                nc.vector.tensor_copy(out=hstage[:, F:], in_=pa[:, 0:8])
                astage = stg.tile([128, 16], DT_BF, tag="a1s")
                nc.vector.tensor_copy(out=astage[:], in_=pa[:])
                nc.sync.dma_start(out=exch1[t * 128:(t + 1) * 128, :], in_=hstage[:])
                nc.sync.dma_start(out=ALD1[t * 128:(t + 1) * 128, :], in_=astage[:])

            rg = [list(range(NC))]
            def exchange(src_t, dst_t):
                if sim_mode:
                    nc.sync.dma_start(out=dst_t[0:SHARD, :], in_=src_t[0:SHARD, :])
                else:
                    nc.gpsimd.collective_compute(
                        "AllGather", ALU.bypass, replica_groups=rg,
                        ins=[src_t[0:SHARD, :].opt()], outs=[dst_t[:].opt()])

            exchange(exch1, Hf1)

            # ---------------- edge phase (used for all 3 layers) -------------
            def edge_phase(layer, Hsrc, ALDsrc, EXL):
                lay3 = layer == 3
                FH = F3 if lay3 else F            # feature width of h
                RW = (F3 + 16) if lay3 else (F + 8)   # gathered row width
                gdt = DT_F32 if lay3 else DT_BF
                CW = C3 if lay3 else C
                als_off = F3 if lay3 else F
                for w in range(W):
                    n0 = w * 128
                    pden = p_B.tile([128, 8], DT_F32, space="PSUM", tag="pden")
                    pout = p_A.tile([128, FH + 8 if lay3 else FH], DT_F32,
                                    space="PSUM", tag="pbig")
                    # window al_d: direct load of 128 consecutive dst rows
                    if lay3:
                        adww = stg.tile([128, F3 + 16], DT_F32, tag="adw3")
                        nc.sync.dma_start(out=adww[:], in_=ALDsrc[n0:n0 + 128, :])
                        adwt = sb.tile([128, 8], DT_BF, tag="adwc")
                        nc.vector.tensor_copy(out=adwt[:], in_=adww[:, F3 + 8:F3 + 16])
                        adw = adwt[:]
                    else:
                        adw16 = stg.tile([128, 16], DT_BF, tag="adwb")
                        nc.sync.dma_start(out=adw16[:], in_=ALDsrc[n0:n0 + 128, :])
                        adw = adw16[:, 8:16]
                    # window gather: K tile-gathers of 128 rows each
                    hg = hgp.tile([128, K, RW], gdt, tag="hg")
                    for k in range(K):
                        nc.gpsimd.indirect_dma_start(
                            out=hg[:, k, :], out_offset=None, in_=Hsrc[:],
                            in_offset=bass.IndirectOffsetOnAxis(
                                ap=t_esrc[:, w * K + k:w * K + k + 1], axis=0))
                    # per-window dst-row table replicated across partitions
                    drT = hgp.tile([128, K, 128], DT_BF, tag="drT")
                    drT_in = _bcast(P["dstrowT"][:],
                                    [[0, 128], [128, K], [1, 128]])
                    drT_in = bass.AP(drT_in.tensor, w * K * 128, drT_in.ap)
                    nc.sync.dma_start(out=drT[:], in_=drT_in)
                    for g0 in range(0, K, G):
                        gn = min(G, K - g0)
                        tbase = w * K + g0
                        # selection matrices + transposes via per-partition
                        # scalar compares (DVE 4x mode)
                        sel = sb.tile([128, G, 128], DT_BF, tag="sel")
                        selT = sb.tile([128, G, 128], DT_BF, tag="selT")
                        pad_ps = p_T.tile([128, G * 8], DT_F32, space="PSUM", tag="pad")
                        for j in range(gn):
                            nc.vector.tensor_scalar(
                                out=sel[:, j, :], in0=iota_b[:],
                                scalar1=t_drow[:, tbase + j:tbase + j + 1],
                                scalar2=None, op0=ALU.is_equal)
                            nc.vector.tensor_scalar(
                                out=selT[:, j, :], in0=drT[:, g0 + j, :],
                                scalar1=iotaP_b[:], scalar2=None,
                                op0=ALU.is_equal)
                            nc.tensor.matmul(pad_ps[:, j * 8:(j + 1) * 8],
                                             lhsT=selT[:, j, :], rhs=adw,
                                             start=True, stop=True)
                        # e = al_s[src] + al_d[dst]; alpha-num = exp(lrelu(e))
                        e_t = sb.tile([128, G, 8], DT_F32, tag="e")
                        pad_v = _bcast(pad_ps[:], [pad_ps[:].ap[0], [8, gn], [1, 8]])
                        nc.vector.tensor_tensor(
                            out=e_t[:, :gn, :],
                            in0=hg[:, g0:g0 + gn, als_off:als_off + 8],
                            in1=pad_v, op=ALU.add)
                        lr = sb.tile([128, G, 8], DT_F32, tag="lr")
                        nc.scalar.activation(lr[:, :gn, :], e_t[:, :gn, :], AF.Lrelu,
                                             alpha=NEG_SLOPE)
                        # expanded exp on ACT: exbig[e,j,h*CW+c] = exp(lr[e,j,h])
                        exbig = sb.tile([128, G, FH], DT_BF, tag="exbig")
                        lr4 = _bcast(lr[:], [lr[:].ap[0], [8, gn], [1, 8], [0, CW]])
                        exb4 = _bcast(exbig[:], [exbig[:].ap[0], [FH, gn],
                                                 [CW, 8], [1, CW]])
                        nc.scalar.activation(exb4, lr4, AF.Exp)
                        # msg = h_gathered * ex  (both step-1 bf16 -> DVE 2x)
                        msg = sb.tile([128, G, FH + 8 if lay3 else FH], DT_BF,
                                      tag="msg")
                        nc.vector.tensor_tensor(out=msg[:, :gn, :FH],
                                                in0=hg[:, g0:g0 + gn, 0:FH],
                                                in1=exbig[:, :gn, :], op=ALU.mult)
                        if lay3:
                            nc.vector.tensor_copy(
                                out=msg[:, :gn, F3:F3 + 8],
                                in_=_bcast(exbig[:], [exbig[:].ap[0], [FH, gn],
                                                      [CW, 8]]))
                        first = g0 == 0
                        for j in range(gn):
                            st = first and j == 0
                            nc.tensor.matmul(pout[:], lhsT=sel[:, j, :],
                                             rhs=msg[:, j, :], start=st, stop=False)
                            if not lay3:
                                ex8 = bass.AP(exbig[:].tensor,
                                              exbig[:, j, :].offset,
                                              [exbig[:].ap[0], [CW, 8]])
                                nc.tensor.matmul(pden[:], lhsT=sel[:, j, :],
                                                 rhs=ex8, start=st, stop=False)
                    # ---- self-loop term from local rows ----
                    es_t = sb.tile([128, 8], DT_F32, tag="est")
                    if lay3:
                        nc.vector.tensor_tensor(out=es_t[:],
                                                in0=adww[:, F3:F3 + 8],
                                                in1=adww[:, F3 + 8:F3 + 16],
                                                op=ALU.add)
                    else:
                        nc.vector.tensor_tensor(out=es_t[:], in0=adw16[:, 0:8],
                                                in1=adw16[:, 8:16], op=ALU.add)
                    lrs = sb.tile([128, 8], DT_F32, tag="lrs")
                    nc.scalar.activation(lrs[:], es_t[:], AF.Lrelu, alpha=NEG_SLOPE)
                    exsE = sb.tile([128, FH], DT_BF, tag="exsE")
                    lrs_b = _bcast(lrs[:], [lrs[:].ap[0], [1, 8], [0, CW]])
                    exsE_b = _bcast(exsE[:], [exsE[:].ap[0], [CW, 8], [1, CW]])
                    nc.scalar.activation(exsE_b, lrs_b, AF.Exp)
                    if lay3:
                        hs_f = stg.tile([128, F3], DT_F32, tag="hs3f")
                        nc.sync.dma_start(out=hs_f[:], in_=EXL[n0:n0 + 128, 0:F3])
                        hself = sb.tile([128, F3], DT_BF, tag="hself")
                        nc.vector.tensor_copy(out=hself[:], in_=hs_f[:])
                    else:
                        hself = stg.tile([128, FH], DT_BF, tag="hself")
                        nc.sync.dma_start(out=hself[:], in_=EXL[n0:n0 + 128, 0:FH])
                    msgs = sb.tile([128, FH + 8 if lay3 else FH], DT_BF, tag="msgs")
                    nc.vector.tensor_tensor(out=msgs[:, :FH], in0=hself[:],
                                            in1=exsE[:], op=ALU.mult)
                    if lay3:
                        nc.vector.tensor_copy(
                            out=msgs[:, F3:F3 + 8],
                            in_=bass.AP(exsE[:].tensor, exsE[:].offset,
                                        [exsE[:].ap[0], [CW, 8]]))
                    nc.tensor.matmul(pout[:], lhsT=ident_bf[:], rhs=msgs[:],
                                     start=False, stop=True)
                    if not lay3:
                        exs8 = bass.AP(exsE[:].tensor, exsE[:].offset,
                                       [exsE[:].ap[0], [CW, 8]])
                        nc.tensor.matmul(pden[:], lhsT=ident_bf[:], rhs=exs8,
                                         start=False, stop=True)
                    # ---- window close: normalize, bias, next-layer ----
                    den = sb.tile([128, 8], DT_F32, tag="den")
                    if lay3:
                        nc.vector.tensor_scalar_add(den[:], pout[:, F3:F3 + 8], 1e-16)
                    else:
                        nc.vector.tensor_scalar_add(den[:], pden[:], 1e-16)
                    rec = sb.tile([128, 8], DT_F32, tag="rec")
                    nc.vector.reciprocal(rec[:], den[:])
                    onrm = sb.tile([128, FH], DT_F32, tag="onrm")
                    rec_b = _bcast(rec[:], [rec[:].ap[0], [1, 8], [0, CW]])
                    po4 = _bcast(pout[:], [pout[:].ap[0], [CW, 8], [1, CW]])
                    on4 = _bcast(onrm[:], [onrm[:].ap[0], [CW, 8], [1, CW]])
                    nc.vector.tensor_tensor(out=on4, in0=po4, in1=rec_b, op=ALU.mult)
                    if lay3:
                        hm = sb.tile([128, C3], DT_F32, tag="hm")
                        on_T = _bcast(onrm[:], [onrm[:].ap[0], [1, C3], [C3, 8]])
                        nc.vector.reduce_sum(hm[:], on_T, axis=mybir.AxisListType.X)
                        nc.vector.tensor_scalar_mul(hm[:], hm[:], 0.125)
                        nc.vector.tensor_add(out=hm[:], in0=hm[:], in1=t_b3[:])
                        mx = sb.tile([128, 1], DT_F32, tag="mx")
                        nc.vector.reduce_max(mx[:], hm[:], axis=mybir.AxisListType.X)
                        xc = sb.tile([128, C3], DT_F32, tag="xc")
                        nc.vector.tensor_tensor(out=xc[:], in0=hm[:],
                                                in1=mx[:].to_broadcast([128, C3]),
                                                op=ALU.subtract)
                        e5 = sb.tile([128, C3], DT_F32, tag="e5")
                        nc.scalar.activation(e5[:], xc[:], AF.Exp)
                        s5 = sb.tile([128, 1], DT_F32, tag="s5")
                        nc.vector.reduce_sum(s5[:], e5[:], axis=mybir.AxisListType.X)
                        lg = sb.tile([128, 1], DT_F32, tag="lg")
                        nc.scalar.activation(lg[:], s5[:], AF.Ln)
                        res = sb.tile([128, C3], DT_F32, tag="res")
                        nc.vector.tensor_tensor(out=res[:], in0=xc[:],
                                                in1=lg[:].to_broadcast([128, C3]),
                                                op=ALU.subtract)
                        nc.sync.dma_start(out=OUTI[n0:n0 + 128, :], in_=res[:])
                        continue
                    # bias + relu -> x_next
                    nc.vector.tensor_add(out=onrm[:], in0=onrm[:],
                                         in1=t_b1[:] if layer == 1 else t_b2[:])
                    xn = sb.tile([128, F], DT_F32, tag="xn")
                    nc.scalar.activation(xn[:], onrm[:], AF.Relu)
                    # transpose x_next -> lhsT chunks
                    xnT = sb.tile([128, 4, 128], DT_BF, tag="xnT")
                    for ch in range(4):
                        pt = p_T.tile([128, 128], DT_F32, space="PSUM", tag="ptx")
                        nc.tensor.transpose(pt[:], xn[:, bass.ts(ch, 128)], ident[:])
                        nc.vector.tensor_copy(out=xnT[:, ch, :], in_=pt[:])
                    # next-layer h / al
                    wN = w2 if layer == 1 else w3
                    wwaN = wwa2 if layer == 1 else wwa3
                    FN = F if layer == 1 else F3
                    ph = p_A.tile([128, FN], DT_F32, space="PSUM", tag="pbig")
                    pa = p_T.tile([128, 16], DT_F32, space="PSUM", tag="pan")
                    for ch in range(4):
                        nc.tensor.matmul(ph[:], lhsT=xnT[:, ch, :], rhs=wN[:, ch, :],
                                         start=(ch == 0), stop=(ch == 3))
                        nc.tensor.matmul(pa[:], lhsT=xnT[:, ch, :], rhs=wwaN[:, ch, :],
                                         start=(ch == 0), stop=(ch == 3))
                    if layer == 1:
                        hstage = stg.tile([128, F + 8], DT_BF, tag="h2s")
                        nc.vector.tensor_copy(out=hstage[:, :F], in_=ph[:])
                        nc.vector.tensor_copy(out=hstage[:, F:], in_=pa[:, 0:8])
                        astage = stg.tile([128, 16], DT_BF, tag="a2s")
                        nc.vector.tensor_copy(out=astage[:], in_=pa[:])
                        nc.sync.dma_start(out=exch2[n0:n0 + 128, :], in_=hstage[:])
                        nc.sync.dma_start(out=ALD2[n0:n0 + 128, :], in_=astage[:])
                    else:
                        h3stage = stg.tile([128, F3 + 16], DT_F32, tag="h3s")
                        nc.vector.tensor_copy(out=h3stage[:, :F3], in_=ph[:])
                        nc.vector.tensor_copy(out=h3stage[:, F3:], in_=pa[:])
                        nc.sync.dma_start(out=exch3[n0:n0 + 128, :], in_=h3stage[:])

            edge_phase(1, Hf1, ALD1, exch1)
            exchange(exch2, Hf2)
            edge_phase(2, Hf2, ALD2, exch2)
            exchange(exch3, H3f)
            edge_phase(3, H3f, exch3, exch3)
            nc.sync.dma_start(out=OUT[:], in_=OUTI[0:SHARD, :])
            tk = sb.tile([128, 1], DT_F32, tag="tick")
            nc.sync.dma_start(out=tk[:], in_=P["tick"][:])
            nc.sync.dma_start(out=TOCK[:], in_=tk[:])

    _split_drain_waits(nc)
    return nc


_CACHE = {}
_last_in_maps = None


def kernel(**inputs):
    x = np.asarray(inputs["x"], np.float32)
    edge_index = np.asarray(inputs["edge_index"], np.int32)
    esrc, dstrow, dstrowT, K, T = host_prep(edge_index)
    key = (K, T)
    if key not in _CACHE:
        _CACHE[key] = build_program(K, T)
    nc = _CACHE[key]

    xT = np.ascontiguousarray(x.T)
    com = {
        "W1": np.asarray(inputs["W1"], np.float32),
        "W1Tc": chunk_rows(np.ascontiguousarray(np.asarray(inputs["W1"]).T)),
        "Wa1": chunk_rows(np.concatenate(
            [blockdiag(np.asarray(inputs["as1"])), blockdiag(np.asarray(inputs["ad1"]))], 1)),
        "W2c": chunk_rows(np.asarray(inputs["W2"], np.float32)),
        "W2Tc": chunk_rows(np.ascontiguousarray(np.asarray(inputs["W2"]).T)),
        "Wa2": chunk_rows(np.concatenate(
            [blockdiag(np.asarray(inputs["as2"])), blockdiag(np.asarray(inputs["ad2"]))], 1)),
        "W3c": chunk_rows(np.asarray(inputs["W3"], np.float32)),
        "W3T": np.ascontiguousarray(np.asarray(inputs["W3"]).T),
        "Wa3": np.concatenate(
            [blockdiag(np.asarray(inputs["as3"])), blockdiag(np.asarray(inputs["ad3"]))], 1),
        "b1t": np.tile(np.asarray(inputs["b1"], np.float32)[None, :], (128, 1)),
        "b2t": np.tile(np.asarray(inputs["b2"], np.float32)[None, :], (128, 1)),
        "b3t": np.tile(np.asarray(inputs["b3"], np.float32)[None, :], (128, 1)),
    }
    in_maps = []
    for c in range(NC):
        m = dict(com)
        m["xTs"] = np.ascontiguousarray(xT[:, c * SHARD:(c + 1) * SHARD])
        m["esrc"] = esrc[c]
        m["dstrow"] = dstrow[c]
        m["dstrowT"] = dstrowT[c]
        m["tick"] = np.zeros((128, 1), np.float32)
        in_maps.append(m)
    global _last_in_maps
    _last_in_maps = in_maps
    res = run_bass_kernel_spmd(nc, in_maps, list(range(NC)))
    return np.concatenate([res.results[c]["out"] for c in range(NC)], axis=0)
